# revision 5
# baseline (speedup 1.0000x reference)
"""Distributed 6-layer GCN (gcn_norm with self-loops) for 8 TRN2 NeuronCores.

Per layer (A_hat = D^-1/2 (A+I) D^-1/2):
    z   = x @ W          (PE, bf16; layer 5 aggregates first, then W5)
    hs  = dinv * z       (f32 table values)
    AllGather hs -> table [131072, 64] f32 in local DRAM
    acc[d] = sum_{s->d} hs[s]   (dma_gather 256B rows + PE bf16 identity-MMs)
    out = dinv * (acc + hs[d]) + b ; leaky_relu(0.2) between layers

Nodes are dealt by in-degree into 8 rank shards of 16384 slots (12500 real).
Gather indices are int16 within 4 address windows of 32768 rows; a greedy
assignment balances each dst's in-edges across windows. Per (dst-tile, window)
edges are laid out in 128-wide rounds padded with a zero row.
"""
import numpy as np

N = 100000
E = 1600000
R = 8
SHARD = 16384
WIN = 32768
NWIN = 4
REAL = N // R                    # 12500
TILES = (REAL + 127) // 128      # 98
NPOS = TILES * 128               # 12544
DIN, DH, DOUT = 128, 64, 4
MAXCOLS_GROUP = 88               # gather-buffer columns per group
CALLCOLS = 62                    # columns per dma_gather call (8 => 1024 idx)
NQUEUES = 4

_cache = {}
_last_maps = None


# ----------------------------------------------------------------- host prep
def _balance_windows(src, dst):
    """Greedy: assign each node a window 0..3 minimizing its out-targets'
    current window counts; capacity 2*REAL real nodes per window."""
    outdeg = np.bincount(src, minlength=N)
    order = np.argsort(-outdeg, kind="stable")
    o = np.argsort(src, kind="stable")
    odst = dst[o]
    optr = np.zeros(N + 1, np.int64)
    np.cumsum(np.bincount(src[o], minlength=N), out=optr[1:])

    counts = np.zeros((NWIN, N), np.int16)
    cap = np.full(NWIN, 2 * REAL, np.int64)
    win = np.full(N, -1, np.int32)
    rng = np.random.default_rng(12345)
    noise = rng.random((N, NWIN)) * 0.25
    for j in range(N):
        u = order[j]
        s, e = optr[u], optr[u + 1]
        tg = odst[s:e]
        if e > s:
            pen = counts[:, tg].sum(axis=1).astype(np.float64)
        else:
            pen = np.zeros(NWIN)
        pen += noise[j]
        pen[cap <= 0] = np.inf
        f = int(np.argmin(pen))
        win[u] = f
        cap[f] -= 1
        if e > s:
            counts[f, tg] += 1
    return win


def _build_graph(edge_index):
    src = edge_index[0].astype(np.int64)
    dst = edge_index[1].astype(np.int64)
    indeg = np.bincount(dst, minlength=N)
    dinv = (1.0 / np.sqrt(indeg + 1.0)).astype(np.float32)

    win = _balance_windows(src, dst)

    # slot assignment: within each window, sort nodes by indeg desc and deal
    # alternately to the window's two ranks; pos = i//2
    slot_of_node = np.full(N, -1, np.int64)
    for f in range(NWIN):
        nf = np.where(win == f)[0]
        nf = nf[np.argsort(-indeg[nf], kind="stable")]
        rk = 2 * f + (np.arange(len(nf)) % 2)
        pos = np.arange(len(nf)) // 2
        assert len(nf) == 0 or pos.max() < REAL
        slot_of_node[nf] = rk * SHARD + pos
    assert (slot_of_node >= 0).all()

    dinv_slot = np.zeros(R * SHARD, np.float32)
    dinv_slot[slot_of_node] = dinv

    sslot = slot_of_node[src]
    dslot = slot_of_node[dst]
    swin = sslot // WIN
    srel = sslot % WIN
    drank = dslot // SHARD
    dpos = dslot % SHARD
    dtile = dpos // 128
    dpart = dpos % 128

    key = ((drank * TILES + dtile) * NWIN + swin) * 128 + dpart
    eorder = np.argsort(key, kind="stable")
    key_s = key[eorder]
    srel_s = srel[eorder]
    cnt = np.bincount(key_s, minlength=R * TILES * NWIN * 128).reshape(
        R, TILES, NWIN, 128)
    K_tf = cnt.max(axis=3).max(axis=0)  # [TILES, NWIN]

    # strided tile order to equalize group loads
    tile_order = [t for g0 in range(25) for t in range(g0, TILES, 25)]
    groups = []
    cur, cur_cols = [], 0
    for t in tile_order:
        c = int(K_tf[t].sum())
        if cur and cur_cols + c > MAXCOLS_GROUP:
            groups.append(cur)
            cur, cur_cols = [], 0
        cur.append(t)
        cur_cols += c
    if cur:
        groups.append(cur)

    colbase = np.zeros((TILES, NWIN), np.int64)
    group_meta = []
    total_cols = 0
    for tl in groups:
        g0 = total_cols
        calls = []
        for f in range(NWIN):
            c0 = total_cols - g0
            nc_ = 0
            for t in tl:
                colbase[t, f] = total_cols
                total_cols += int(K_tf[t, f])
                nc_ += int(K_tf[t, f])
            off = 0
            while off < nc_:
                n = min(CALLCOLS, nc_ - off)
                calls.append((f, (g0 + c0 + off) * 128, n * 128, c0 + off, n))
                off += n
        segs = {t: [(int(colbase[t, f]) - g0, int(K_tf[t, f]))
                    for f in range(NWIN) if K_tf[t, f] > 0] for t in tl}
        group_meta.append(dict(cols=total_cols - g0, tiles=list(tl),
                               calls=calls, segs=segs, col0=g0))
    NIDX = total_cols * 128

    zero_rel = NPOS + 8  # a zero row inside every window (rank 2f pad pos)

    idx = np.full((R, total_cols, 128), zero_rel, np.int64)
    ptr = np.zeros(R * TILES * NWIN * 128 + 1, np.int64)
    np.cumsum(cnt.ravel(), out=ptr[1:])
    rib = np.arange(E) - ptr[key_s]
    er = key_s // (TILES * NWIN * 128)
    erem = key_s % (TILES * NWIN * 128)
    et = erem // (NWIN * 128)
    ef = (erem // 128) % NWIN
    ep = erem % 128
    ecol = colbase[et, ef] + rib
    idx[er, ecol, ep] = srel_s

    idx_flat = idx.reshape(R, NIDX)
    wrapped = idx_flat.reshape(R, NIDX // 16, 16).transpose(0, 2, 1)
    idx_tiles = np.tile(wrapped, (1, 8, 1)).astype(np.int16)

    pos_grid = np.arange(NPOS).reshape(TILES, 128)
    dinv_rt = np.zeros((R, 128, TILES), np.float32)
    for r in range(R):
        d = np.where(pos_grid < REAL, dinv_slot[r * SHARD + pos_grid], 0.0)
        dinv_rt[r] = d.T
    return dict(
        slot_of_node=slot_of_node, idx_tiles=idx_tiles, dinv_rt=dinv_rt,
        groups=group_meta, NIDX=NIDX, total_cols=total_cols,
        padding=NIDX * R / float(E),
    )


# ------------------------------------------------------------- bass program
def _build_program(meta, with_bias):
    from concourse import bacc, bass, mybir, tile
    from concourse.masks import make_identity

    NIDX = meta["NIDX"]
    BF = mybir.dt.bfloat16
    F32 = mybir.dt.float32
    nc = bacc.Bacc("TRN2", target_bir_lowering=False, debug=False,
                   num_devices=R, num_swdge_queues=NQUEUES)

    xs_d = nc.dram_tensor("xs", [SHARD, DIN], F32, kind="ExternalInput")
    gidx_d = nc.dram_tensor("gidx", [128, NIDX // 16], mybir.dt.int16,
                            kind="ExternalInput")
    dinv_d = nc.dram_tensor("dinv", [128, TILES], F32, kind="ExternalInput")
    W_d = [nc.dram_tensor(f"W{i}", [DIN if i == 0 else DH,
                                    DOUT if i == 5 else DH],
                          F32, kind="ExternalInput") for i in range(6)]
    if with_bias:
        bb_d = nc.dram_tensor("bb", [5 * 128, DH], F32, kind="ExternalInput")
    out_d = nc.dram_tensor("out", [NPOS, DOUT], F32, kind="ExternalOutput")

    ag_in = nc.dram_tensor("ag_in", [SHARD, DH], F32)
    table = nc.dram_tensor("table", [R * SHARD, DH], F32)

    AL = mybir.AluOpType
    with tile.TileContext(nc) as tc:
        with (
            tc.tile_pool(name="const", bufs=1) as constp,
            tc.tile_pool(name="persist", bufs=1) as persist,
            tc.tile_pool(name="x0p", bufs=3) as x0p,
            tc.tile_pool(name="xtp", bufs=3) as xtp,
            tc.tile_pool(name="gbuf", bufs=2) as gbuf,
            tc.tile_pool(name="gbb", bufs=2) as gbb,
            tc.tile_pool(name="ep", bufs=4) as epp,
            tc.tile_pool(name="ps_acc", bufs=3, space="PSUM") as ps_acc,
            tc.tile_pool(name="ps_tr", bufs=2, space="PSUM") as ps_tr,
            tc.tile_pool(name="ps_h", bufs=2, space="PSUM") as ps_h,
        ):
            identb = constp.tile([128, 128], BF)
            make_identity(nc, identb[:])
            idx_t = persist.tile([128, NIDX // 16], mybir.dt.int16)
            nc.sync.dma_start(out=idx_t[:], in_=gidx_d[:])
            dinv_t = constp.tile([128, TILES], F32)
            nc.sync.dma_start(out=dinv_t[:], in_=dinv_d[:])
            W_t = []
            for i in range(6):
                wt = constp.tile(list(W_d[i].shape), BF, tag=f"W{i}")
                nc.gpsimd.dma_start(out=wt[:], in_=W_d[i][:])  # casts f32->bf16
                W_t.append(wt)
            if with_bias:
                bb_t = constp.tile([128, 5 * DH], F32)
                nc.sync.dma_start(
                    out=bb_t[:].rearrange("p (l d) -> p l d", d=DH),
                    in_=bb_d[:].rearrange("(l p) d -> p l d", p=128))

            hs_buf = persist.tile([128, TILES * DH], F32)
            xa = persist.tile([128, TILES * DH], BF)
            xb = persist.tile([128, TILES * DH], BF)
            out_sb = persist.tile([128, TILES * DOUT], F32)

            zpad = SHARD - NPOS
            zt = persist.tile([128, (zpad // 128) * DH], F32)
            nc.gpsimd.memset(zt[:], 0.0)
            nc.sync.dma_start(
                out=ag_in[NPOS:SHARD, :].rearrange("(t p) d -> p t d", p=128),
                in_=zt[:].rearrange("p (t d) -> p t d", d=DH))

            def dinv_col(t):
                return dinv_t[:, t:t + 1].to_broadcast([128, DH])

            qn = [0]

            def next_q():
                q = qn[0] % NQUEUES
                qn[0] += 1
                return q

            for l in range(6):
                xsrc = xa if l % 2 == 1 else xb
                xdst = xb if l % 2 == 1 else xa
                for t in range(TILES):
                    if l == 0:
                        x0t = x0p.tile([128, DIN], BF, tag="x0")
                        nc.gpsimd.dma_start(  # f32 -> bf16 cast on load
                            out=x0t[:], in_=xs_d[t * 128:(t + 1) * 128, :])
                        tr = ps_tr.tile([128, 128], BF, space="PSUM", tag="tr")
                        nc.tensor.transpose(out=tr[:], in_=x0t[:],
                                            identity=identb[:])
                        xT = xtp.tile([128, 128], BF, tag="xT")
                        nc.vector.tensor_copy(out=xT[:], in_=tr[:])
                        h = ps_h.tile([128, DH], F32, space="PSUM", tag="h")
                        nc.tensor.matmul(out=h[:], lhsT=xT[:], rhs=W_t[0][:],
                                         start=True, stop=True)
                        nc.vector.tensor_tensor(
                            out=hs_buf[:, t * DH:(t + 1) * DH],
                            in0=h[:], in1=dinv_col(t), op=AL.mult)
                    elif l <= 4:
                        xt_ap = xsrc[:, t * DH:(t + 1) * DH]
                        tr = ps_tr.tile([128, 128], BF, space="PSUM", tag="tr")
                        nc.tensor.transpose(out=tr[:DH, :], in_=xt_ap,
                                            identity=identb[:])
                        xT = xtp.tile([128, 128], BF, tag="xT")
                        nc.vector.tensor_copy(out=xT[:DH, :], in_=tr[:DH, :])
                        h = ps_h.tile([128, DH], F32, space="PSUM", tag="h")
                        nc.tensor.matmul(out=h[:], lhsT=xT[:DH, :],
                                         rhs=W_t[l][:], start=True, stop=True)
                        nc.vector.tensor_tensor(
                            out=hs_buf[:, t * DH:(t + 1) * DH],
                            in0=h[:], in1=dinv_col(t), op=AL.mult)
                    else:
                        nc.vector.tensor_tensor(
                            out=hs_buf[:, t * DH:(t + 1) * DH],
                            in0=xsrc[:, t * DH:(t + 1) * DH],
                            in1=dinv_col(t), op=AL.mult)
                nc.sync.dma_start(
                    out=ag_in[:NPOS, :].rearrange("(t p) d -> p t d", p=128),
                    in_=hs_buf[:].rearrange("p (t d) -> p t d", d=DH))
                nc.gpsimd.collective_compute(
                    "AllGather", AL.bypass,
                    replica_groups=[list(range(R))],
                    ins=[ag_in[:]], outs=[table[:]])
                for gm in meta["groups"]:
                    C = gm["cols"]
                    gt = gbuf.tile([128, C, DH], F32, tag="g")
                    for (f, ioff, ni, c0, ncols) in gm["calls"]:
                        nc.gpsimd.dma_gather(
                            out_ap=gt[:, c0:c0 + ncols, :],
                            in_ap=table[f * WIN:(f + 1) * WIN, :],
                            idxs_ap=idx_t[:16, ioff // 16:(ioff + ni) // 16],
                            num_idxs=ni, num_idxs_reg=ni, elem_size=DH,
                            single_packet=(ncols <= 8), queue_num=next_q())
                    gbt = gbb.tile([128, C, DH], BF, tag="gb")
                    nc.vector.tensor_copy(out=gbt[:], in_=gt[:])
                    for t in gm["tiles"]:
                        acc = ps_acc.tile([128, DH], F32, space="PSUM",
                                          tag="acc")
                        segs = gm["segs"][t]
                        nk = sum(s[1] for s in segs)
                        ki = 0
                        for (c0, ncols) in segs:
                            for k in range(ncols):
                                nc.tensor.matmul(
                                    out=acc[:], lhsT=identb[:],
                                    rhs=gbt[:, c0 + k, :],
                                    start=(ki == 0), stop=(ki == nk - 1))
                                ki += 1
                        tmp = epp.tile([128, DH], F32, tag="tmp")
                        nc.vector.tensor_tensor(
                            out=tmp[:], in0=acc[:],
                            in1=hs_buf[:, t * DH:(t + 1) * DH], op=AL.add)
                        if l < 5:
                            a2 = epp.tile([128, DH], F32, tag="a2")
                            nc.vector.tensor_tensor(
                                out=a2[:], in0=tmp[:], in1=dinv_col(t),
                                op=AL.mult)
                            if with_bias:
                                nc.vector.tensor_tensor(
                                    out=a2[:], in0=a2[:],
                                    in1=bb_t[:, l * DH:(l + 1) * DH],
                                    op=AL.add)
                            t3 = epp.tile([128, DH], F32, tag="t3")
                            nc.scalar.mul(out=t3[:], in_=a2[:], mul=0.2)
                            nc.vector.tensor_tensor(
                                out=xdst[:, t * DH:(t + 1) * DH],
                                in0=a2[:], in1=t3[:], op=AL.max)
                        else:
                            agg = epp.tile([128, DH], BF, tag="a2")
                            nc.vector.tensor_tensor(
                                out=agg[:], in0=tmp[:], in1=dinv_col(t),
                                op=AL.mult)
                            tr = ps_tr.tile([128, 128], BF, space="PSUM",
                                            tag="tr")
                            nc.tensor.transpose(out=tr[:DH, :], in_=agg[:],
                                                identity=identb[:])
                            aggT = xtp.tile([128, 128], BF, tag="xT")
                            nc.vector.tensor_copy(out=aggT[:DH, :],
                                                  in_=tr[:DH, :])
                            o5 = ps_h.tile([128, DOUT], F32, space="PSUM",
                                           tag="h")
                            nc.tensor.matmul(out=o5[:], lhsT=aggT[:DH, :],
                                             rhs=W_t[5][:], start=True,
                                             stop=True)
                            nc.vector.tensor_copy(
                                out=out_sb[:, t * DOUT:(t + 1) * DOUT],
                                in_=o5[:])
            nc.sync.dma_start(
                out=out_d[:].rearrange("(t p) d -> p t d", p=128),
                in_=out_sb[:].rearrange("p (t d) -> p t d", d=DOUT))
    nc.compile()
    return nc


# ------------------------------------------------------------------ runner
def kernel(**inputs):
    from concourse.bass_utils import run_bass_kernel_spmd

    edge_index = np.asarray(inputs["edge_index"])
    x = np.asarray(inputs["x"], dtype=np.float32)
    Ws = [np.asarray(inputs[f"W{i}"], dtype=np.float32) for i in range(6)]
    bs = [np.asarray(inputs[f"b{i}"], dtype=np.float32) for i in range(6)]
    with_bias = any(float(np.abs(b).max()) > 0 for b in bs[:5])

    ck = ("prog", edge_index.shape[1], with_bias,
          int(edge_index[0, :8].sum()), int(edge_index[1, :8].sum()))
    if ck not in _cache:
        meta = _build_graph(edge_index)
        nc = _build_program(meta, with_bias)
        _cache[ck] = (meta, nc)
    meta, nc = _cache[ck]

    xs = np.zeros((R * SHARD, DIN), np.float32)
    xs[meta["slot_of_node"]] = x
    xs = xs.reshape(R, SHARD, DIN)

    maps = []
    for r in range(R):
        m = {
            "xs": xs[r],
            "gidx": meta["idx_tiles"][r],
            "dinv": meta["dinv_rt"][r],
        }
        for i in range(6):
            m[f"W{i}"] = Ws[i]
        if with_bias:
            m["bb"] = np.repeat(np.stack(bs[:5])[:, None, :], 128, axis=1
                                ).reshape(5 * 128, DH).astype(np.float32)
        maps.append(m)

    global _last_maps
    _last_maps = maps
    res = run_bass_kernel_spmd(nc, maps, core_ids=list(range(R)))
    out_full = np.zeros((R * SHARD, DOUT), np.float32)
    for r in range(R):
        out_full[r * SHARD:r * SHARD + NPOS] = res.results[r]["out"]
    out = out_full[meta["slot_of_node"]]
    if float(np.abs(bs[5]).max()) > 0:
        out = out + bs[5][None, :]
    return out.astype(np.float32)
